# revision 48
# baseline (speedup 1.0000x reference)
"""Trainium2 Bass kernel for segmented linear multi-head attention (no softmax).

Reference computation (H=16 heads, T=2048, D=1024, Dh=64):
    qkv = x @ w_qkv + b_qkv -> q,k,v (H,T,Dh)
    mask[t,s] = causal & same-segment (segments cut by `done` resets)
    out = (mask * q k^T) v + (q * before_first) @ state
    new_state = (done_any ? 0 : state) + k_aft^T v_aft   (aft = after last reset)
    x_out = out.reshape(T,D) @ w_out + b_out

Strategy: tensor-parallel over heads, 2 heads per core on 8 cores. Each core:
  - transposes x on-chip (PE transpose) -> xT
  - projects qT,kT (head-dim on partitions) and k,v (token-dim on partitions)
  - runs a chunked scan over 8 chunks of 256 tokens: masked intra-chunk
    attention (block-diagonal only) + a running (64,64) per-head state S
    carrying cross-chunk same-segment contributions. Segment logic
    (masks / flags / keep scalars) is precomputed on host from `done` and fed
    as runtime data, so the instruction stream is static.
  - output-projects its 2 heads into a partial (T,D) sum; host adds partials.
The chunked form makes new_state = final S, exactly matching the reference.

All large matmuls use the FP32R mode (fp32 bits, single-pass reduced-precision
multiply, 4x the fp32 PE rate at N>=256; measured ~1.6e-4 rel err per K=128
contraction on HW). End-to-end kernel error vs the fp32 reference: ~5e-4.
"""

import numpy as np
import sys

for _p in ("/opt/trn_rl_repo", "/opt/trn_rl_repo/concourse"):
    if _p not in sys.path:
        sys.path.insert(0, _p)

H, T, D = 16, 2048, 1024
DH = D // H          # 64
CB = 128             # token sub-block (partition tile)
NB = T // CB         # 16 sub-blocks
CC = 256             # scan chunk (2 sub-blocks)
NC_ = T // CC        # 8 chunks
ND = D // 128        # 8 contract-dim blocks
N_CORES = 8
NHL = H // N_CORES   # 2 heads per core

_NC = None           # cached Bass program


def _build_nc(reps=1):
    import concourse.mybir as mybir
    import concourse.tile as tile
    from concourse import bacc
    from concourse.masks import make_identity
    import contextlib

    fp32 = mybir.dt.float32
    f32r = mybir.dt.float32r
    mul = mybir.AluOpType.mult
    add = mybir.AluOpType.add
    # Bacc (not raw Bass): its compile pipeline legalizes multi-semaphore
    # waits (move_matmul_waits_to_ldweights + generate_event_semaphores),
    # which TRN2 instructions need — each can encode only one wait.
    nc = bacc.Bacc(None)

    def r(ap):  # FP32R view: same bits, tags producer/operand as f32r
        return ap.bitcast(f32r)

    # ---- DRAM I/O (per-core tensors; host pre-shapes everything 2D) ----
    x_d = nc.dram_tensor("x", [T, D], fp32, kind="ExternalInput")
    wqk_d = nc.dram_tensor("wqk", [128, ND * 256], fp32, kind="ExternalInput")
    wkv_d = nc.dram_tensor("wkv", [128, ND * 256], fp32, kind="ExternalInput")
    wout_d = nc.dram_tensor("wout", [128, D], fp32, kind="ExternalInput")
    bqk_d = nc.dram_tensor("bqk", [1, 256], fp32, kind="ExternalInput")
    bkv_d = nc.dram_tensor("bkv", [1, 256], fp32, kind="ExternalInput")
    st_d = nc.dram_tensor("state0", [128, DH], fp32, kind="ExternalInput")
    cumT_d = nc.dram_tensor("cumT", [1, T], fp32, kind="ExternalInput")
    # meta cols: 0:16 flagL | 16:24 keep | 24:40 cumS | 40:48 segIn
    meta_d = nc.dram_tensor("meta", [128, 48], fp32, kind="ExternalInput")
    outp_d = nc.dram_tensor("out_partial", [T, D], fp32, kind="ExternalOutput")
    sout_d = nc.dram_tensor("s_out", [128, DH], fp32, kind="ExternalOutput")

    with tile.TileContext(nc) as tc:
        ctx = contextlib.ExitStack()
        with ctx:
            if reps > 1:  # timing mode: repeat the whole body on-device
                ctx.enter_context(tc.For_i(0, reps, 1))
            cpool = ctx.enter_context(tc.tile_pool(name="const", bufs=1))
            wpool = ctx.enter_context(tc.tile_pool(name="work", bufs=3))
            xpool = ctx.enter_context(tc.tile_pool(name="xstage", bufs=6))
            maskp = ctx.enter_context(tc.tile_pool(name="maskp", bufs=4))
            mmp = ctx.enter_context(tc.tile_pool(name="mmp", bufs=3, space="PSUM"))
            atp = ctx.enter_context(tc.tile_pool(name="atp", bufs=3, space="PSUM"))
            ssp = ctx.enter_context(tc.tile_pool(name="ssp", bufs=2, space="PSUM"))

            # ---- persistent SBUF tiles ----
            ident = cpool.tile([128, 128], fp32)
            wqk_t = cpool.tile([128, ND * 256], fp32)
            wkv_t = cpool.tile([128, ND * 256], fp32)
            wout_t = cpool.tile([128, D], fp32)
            bqk_t = cpool.tile([1, 256], fp32)
            bkv_t = cpool.tile([1, 256], fp32)
            ones_t = cpool.tile([1, 512], fp32)
            tri_t = cpool.tile([128, 512], fp32)
            cumT_t = cpool.tile([1, T], fp32)
            meta_t = cpool.tile([128, 48], fp32)
            S_t = cpool.tile([128, DH], fp32)
            xT_t = cpool.tile([128, ND * T], fp32)      # [d%128, (d//128)*T + t]
            qkT_t = cpool.tile([128, 2 * T], fp32)      # [hdh, t] q | [hdh, T+t] k
            kvn_t = cpool.tile([128, NB * 256], fp32)   # [t%128, b*256 + (k0|k1|v0|v1)]
            outT_t = cpool.tile([128, T], fp32)         # [hdh, t]

            nc.gpsimd.memset(ones_t[:], 1.0)
            # causal (upper-triangular in (s,t)) masks for the two s-halves of
            # a 256-token chunk: tri[p, half*256+t] = (t >= 128*half + p)
            nc.gpsimd.memset(tri_t[:], 1.0)
            for half in range(2):
                nc.gpsimd.affine_select(
                    out=tri_t[:, half * 256:(half + 1) * 256],
                    in_=tri_t[:, half * 256:(half + 1) * 256],
                    compare_op=mybir.AluOpType.is_ge,
                    fill=0.0,
                    base=-128 * half,
                    pattern=[[1, 256]],
                    channel_multiplier=-1,
                )
            make_identity(nc, ident[:])
            # PE prewarm: first PE op depends only on the GpSimd-built identity,
            # keeping later PE ops at <=1 new semaphore each.
            dmy = mmp.tile([128, 128], fp32, tag="mm")
            nc.tensor.transpose(dmy[:], ident[:], ident[:])

            dma = nc.sync.dma_start

            # ---- Phase A interleaved with const DMAs ----
            # x sub-block loads are emitted FIRST so the serial DMA dispatch
            # stream doesn't stall the PE transposes behind 5MB of constants.
            xs_tiles = {}

            def load_xs(i):
                xs = xpool.tile([128, D], fp32, tag="xs")
                dma(out=xs[:], in_=x_d[i * CB:(i + 1) * CB, :])
                xs_tiles[i] = xs

            mask_tiles = {}
            flagF_tiles = {}

            def gen_mask_chunk(c):  # build maskT + flagF for chunk c on GpSimd
                # (Pool engine is otherwise idle; everything SBUF-only here.)
                bc = wpool.tile([128, CC], fp32, tag="bc")  # cum broadcast to rows
                nc.gpsimd.partition_broadcast(bc[:], cumT_t[0:1, c * CC:(c + 1) * CC])
                ff = maskp.tile([128, CC], fp32, tag="fchunk")
                nc.gpsimd.tensor_scalar(  # flagF: token in carried-in segment
                    ff[:], bc[:],
                    meta_t[:, 40 + c:41 + c], None, mybir.AluOpType.is_equal)
                flagF_tiles[c] = ff
                mk = maskp.tile([128, 512], fp32, tag="mchunk")
                for half in range(2):
                    b = 2 * c + half
                    eq = wpool.tile([128, CC], fp32, tag="eqm")
                    nc.gpsimd.tensor_scalar(eq[:], bc[:], meta_t[:, 24 + b:25 + b],
                                            None, mybir.AluOpType.is_equal)
                    nc.gpsimd.tensor_tensor(
                        mk[:, half * 256:(half + 1) * 256],
                        eq[:], tri_t[:, half * 256:(half + 1) * 256], mul)
                mask_tiles[c] = mk

            # DMA dispatch is serial per queue-stream; order to match first use:
            # x blocks + wkv (phase C) first, wqk (B), wout (E).
            load_xs(0)
            load_xs(1)
            dma(out=S_t[:].bitcast(f32r), in_=st_d[:].bitcast(f32r))
            dma(out=bqk_t[:], in_=bqk_d[:])
            dma(out=bkv_t[:], in_=bkv_d[:])
            dma(out=cumT_t[:], in_=cumT_d[:])
            dma(out=meta_t[:], in_=meta_d[:])
            half_w = ND * 128
            dma(out=wkv_t[:, :half_w].bitcast(f32r), in_=wkv_d[:, :half_w].bitcast(f32r))
            dma(out=wkv_t[:, half_w:].bitcast(f32r), in_=wkv_d[:, half_w:].bitcast(f32r))
            load_xs(2)
            dma(out=wqk_t[:, :half_w].bitcast(f32r), in_=wqk_d[:, :half_w].bitcast(f32r))
            dma(out=wqk_t[:, half_w:].bitcast(f32r), in_=wqk_d[:, half_w:].bitcast(f32r))
            load_xs(3)
            gen_mask_chunk(0)
            gen_mask_chunk(1)
            dma(out=wout_t[:].bitcast(f32r), in_=wout_d[:].bitcast(f32r))

            xT_v = xT_t[:].rearrange("p (j t) -> p j t", j=ND)

            def phase_a(i):  # transpose x sub-block i into xT
                xs = xs_tiles.pop(i)
                for jg in range(2):
                    tp = mmp.tile([128, 512], fp32, tag="mm")
                    for jj in range(4):
                        j = jg * 4 + jj
                        nc.tensor.transpose(
                            tp[:, jj * 128:(jj + 1) * 128],
                            xs[:, j * 128:(j + 1) * 128],
                            ident[:],
                        )
                    nc.vector.tensor_copy(
                        r(xT_v[:, jg * 4:(jg + 1) * 4, i * CB:(i + 1) * CB]),
                        r(tp[:].rearrange("p (j t) -> p j t", j=4)),
                    )

            def phase_b(m, n):  # qT / kT projection, one 512-token chunk
                ps = mmp.tile([128, 512], fp32, tag="mm")
                for j in range(ND):
                    nc.tensor.matmul(
                        ps[:],
                        lhsT=r(wqk_t[:, j * 256 + m * 128: j * 256 + (m + 1) * 128]),
                        rhs=r(xT_t[:, j * T + n * 512: j * T + (n + 1) * 512]),
                        start=(j == 0), stop=False,
                    )
                nc.tensor.matmul(  # + bias (rank-1 with ones row)
                    ps[:],
                    lhsT=bqk_t[0:1, m * 128:(m + 1) * 128],
                    rhs=ones_t[0:1, 0:512],
                    start=False, stop=True,
                )
                nc.vector.tensor_copy(
                    r(qkT_t[:, m * T + n * 512: m * T + (n + 1) * 512]), r(ps[:]))

            def phase_c(b):  # k / v natural projection, one 128-token sub-block
                ps = mmp.tile([128, 256], fp32, tag="mm")
                for j in range(ND):
                    nc.tensor.matmul(
                        ps[:],
                        lhsT=r(xT_t[:, j * T + b * CB: j * T + (b + 1) * CB]),
                        rhs=r(wkv_t[:, j * 256:(j + 1) * 256]),
                        start=(j == 0), stop=False,
                    )
                nc.tensor.matmul(
                    ps[:],
                    lhsT=ones_t[0:1, 0:128],
                    rhs=bkv_t[0:1, :],
                    start=False, stop=True,
                )
                nc.scalar.copy(r(kvn_t[:, b * 256:(b + 1) * 256]), r(ps[:]))

            def ksub(b, h):  # k columns for sub-block b, head h
                return kvn_t[:, b * 256 + 64 * h: b * 256 + 64 * h + 64]

            def vsub(b, h):
                return kvn_t[:, b * 256 + 128 + 64 * h: b * 256 + 128 + 64 * h + 64]

            def phase_d(i):  # chunked segmented attention scan, chunk i (256 tok)
                csl = slice(i * CC, (i + 1) * CC)
                qf = wpool.tile([128, CC], fp32, tag="qf")  # flagF-gated qT
                nc.vector.tensor_tensor(r(qf[:]), r(qkT_t[:, csl]),
                                        r(flagF_tiles.pop(i)[:]), mul)
                # all 4 score matmuls first, then masks, then per-head outputs:
                # keeps PE streaming while DVE masks trail one tile behind.
                scs = {}
                for h in range(NHL):
                    for half in range(2):
                        sc = atp.tile([128, CC], fp32, tag="sc")   # scoresT rows s-half
                        nc.tensor.matmul(
                            sc[:],
                            lhsT=r(qkT_t[64 * h:64 * h + 64,
                                         T + i * CC + half * 128: T + i * CC + half * 128 + 128]),
                            rhs=r(qkT_t[64 * h:64 * h + 64, csl]),
                            start=True, stop=True,
                        )
                        scs[h, half] = sc
                mk = mask_tiles.pop(i)
                ats = {}
                for h in range(NHL):
                    for half in range(2):
                        at = wpool.tile([128, CC], fp32, tag=f"at_sb{half}")
                        nc.vector.tensor_tensor(
                            r(at[:]), r(scs[h, half][:]),
                            r(mk[:, half * 256:(half + 1) * 256]),
                            mul)
                        ats[h, half] = at
                for h in range(NHL):
                    S_sl = S_t[64 * h:64 * h + 64, :]
                    oT = ssp.tile([DH, CC], fp32, tag="small")     # outT chunk
                    nc.tensor.matmul(oT[:], lhsT=r(vsub(2 * i, h)), rhs=r(ats[h, 0][:]),
                                     start=True, stop=False)
                    nc.tensor.matmul(oT[:], lhsT=r(vsub(2 * i + 1, h)), rhs=r(ats[h, 1][:]),
                                     start=False, stop=False)
                    nc.tensor.matmul(oT[:], lhsT=r(S_sl), rhs=r(qf[64 * h:64 * h + 64, :]),
                                     start=False, stop=True)
                    nc.vector.tensor_copy(r(outT_t[64 * h:64 * h + 64, csl]), r(oT[:]))
                for h in range(NHL):
                    S_sl = S_t[64 * h:64 * h + 64, :]
                    sp_ = ssp.tile([DH, DH], fp32, tag="small")
                    for half in range(2):
                        b = 2 * i + half
                        vf = wpool.tile([CB, DH], fp32, tag="vf")  # flagL-gated v
                        nc.vector.tensor_scalar(vf[:], vsub(b, h),
                                                meta_t[:, b:b + 1], None, mul)
                        nc.tensor.matmul(sp_[:], lhsT=ksub(b, h), rhs=vf[:],
                                         start=(half == 0), stop=(half == 1))
                    # S <- keep*S + k^T v  (in-place; Tile orders vs the reads above)
                    nc.vector.tensor_scalar(r(S_sl), S_sl,
                                            meta_t[64 * h:64 * h + 64, 16 + i:17 + i], None, mul)
                    nc.vector.tensor_tensor(r(S_sl), r(S_sl), r(sp_[:]), add)

            def phase_e(b):  # output projection + store, one 128-token sub-block
                os_ = wpool.tile([128, D], fp32, tag="os")
                for n in range(2):
                    pe_ = mmp.tile([128, 512], fp32, tag="mm")
                    nc.tensor.matmul(
                        pe_[:],
                        lhsT=r(outT_t[:, b * CB:(b + 1) * CB]),
                        rhs=r(wout_t[:, n * 512:(n + 1) * 512]),
                        start=True, stop=True,
                    )
                    nc.scalar.copy(os_[:, n * 512:(n + 1) * 512], pe_[:])
                dma(out=outp_d[b * CB:(b + 1) * CB, :], in_=os_[:])

            # Interleaved emission: PE executes its stream in order, so emit
            # each consumer as soon as its producers exist. Per group of 4
            # x-sub-blocks: transposes+kv-projections, then the q/k 512-chunk,
            # then two attention chunks, each followed by its two output
            # projections (which start the big output DMAs early).
            for g in range(4):
                for i in range(4 * g, 4 * g + 4):
                    nxt = i + 4
                    if nxt < NB and nxt not in xs_tiles:
                        load_xs(nxt)
                    phase_a(i)
                    phase_c(i)
                if g < 3:  # masks for next group's attention chunks
                    gen_mask_chunk(2 * g + 2)
                    gen_mask_chunk(2 * g + 3)
                phase_b(0, g)
                phase_b(1, g)
                for c in (2 * g, 2 * g + 1):
                    phase_d(c)
                    phase_e(2 * c)
                    phase_e(2 * c + 1)

            dma(out=sout_d[:], in_=S_t[:])

    # Run Bacc's compile pipeline (register alloc + sync-wait legalization)
    # and freeze; run_bass_via_pjrt ships the module as-is.
    nc.finalize()
    return nc


def _host_prep(done):
    """Segment bookkeeping from done flags (tiny, pure numpy).
    Returns cumT (1,T) and meta (128,48): flagL | keep | cumS | segIn."""
    cum = np.cumsum(np.asarray(done).astype(np.int64)).astype(np.float32)
    meta = np.zeros((128, 48), np.float32)
    for i in range(NC_):
        cb = cum[i * CC:(i + 1) * CC]
        seg_in = 0.0 if i == 0 else cum[i * CC - 1]
        for half in range(2):
            b = 2 * i + half
            meta[:, b] = (cb[half * 128:(half + 1) * 128] == cb[-1])  # flagL
            meta[:, 24 + b] = cb[half * 128:(half + 1) * 128]         # cumS
        meta[:, 16 + i] = 1.0 if cb[-1] == seg_in else 0.0            # keep
        meta[:, 40 + i] = seg_in                                      # segIn
    return cum[None, :].copy(), meta


def make_in_maps(state, x, done, w_qkv, b_qkv, w_out, b_out):
    """Per-core input dicts (host-side sharding + packing)."""
    state = np.ascontiguousarray(np.asarray(state, np.float32))
    x = np.ascontiguousarray(np.asarray(x, np.float32))
    w_qkv = np.asarray(w_qkv, np.float32)
    b_qkv = np.asarray(b_qkv, np.float32)
    w_out = np.asarray(w_out, np.float32)
    cumT, meta = _host_prep(done)
    in_maps = []
    for c in range(N_CORES):
        hs = [NHL * c + h for h in range(NHL)]
        cols = np.concatenate([np.arange(h * DH, (h + 1) * DH) for h in hs])
        wq, wk, wv = w_qkv[:, cols], w_qkv[:, D + cols], w_qkv[:, 2 * D + cols]
        bq, bk, bv = b_qkv[cols], b_qkv[D + cols], b_qkv[2 * D + cols]
        # SBUF layouts: wqk[p, j*256 + (q:0..128 | k:128..256)], j = d//128
        wqk = np.concatenate([wq, wk], axis=1).reshape(ND, 128, 256)
        wqk = np.ascontiguousarray(wqk.transpose(1, 0, 2).reshape(128, ND * 256))
        wkv = np.concatenate([wk, wv], axis=1).reshape(ND, 128, 256)
        wkv = np.ascontiguousarray(wkv.transpose(1, 0, 2).reshape(128, ND * 256))
        in_maps.append({
            "x": x,
            "wqk": wqk,
            "wkv": wkv,
            "wout": np.ascontiguousarray(w_out[cols]),
            "bqk": np.ascontiguousarray(np.concatenate([bq, bk])[None, :]),
            "bkv": np.ascontiguousarray(np.concatenate([bk, bv])[None, :]),
            "state0": np.ascontiguousarray(state[hs].reshape(128, DH)),
            "cumT": cumT,
            "meta": meta,
        })
    return in_maps


def combine_results(results, b_out):
    x_out = np.zeros((T, D), np.float32)
    new_state = np.zeros((H, DH, DH), np.float32)
    for c, r in enumerate(results):
        x_out += r["out_partial"]
        new_state[NHL * c:NHL * (c + 1)] = r["s_out"].reshape(NHL, DH, DH)
    x_out += np.asarray(b_out, np.float32)[None, :]
    return new_state, x_out


def kernel(state, x, done, w_qkv, b_qkv, w_out, b_out):
    global _NC
    from concourse.bass_utils import run_bass_kernel_spmd

    if _NC is None:
        _NC = _build_nc()
    in_maps = make_in_maps(state, x, done, w_qkv, b_qkv, w_out, b_out)
    res = run_bass_kernel_spmd(_NC, in_maps, core_ids=list(range(N_CORES)))
    return combine_results(res.results, b_out)
